# revision 9
# baseline (speedup 1.0000x reference)
"""StyleGAN2-style modulated 3x3 conv (B=8, Ci=Co=512, H=W=32) on 8 TRN2 NeuronCores.

Sharding: data-parallel over batch, one sample per core. Per core the conv
is 9 shifted matmuls over a zero-padded 34x34 image in SBUF, contracting
over Ci in 128-chunks with fp32 PSUM accumulation; compute dtype bf16.

Math (per sample b, with s = (Ci*K*K)**-0.5 folded out of both the conv
and the demod norm so the weights can be used unscaled):
  conv = conv2d(x * y_s, weight)                     # raw, no s
  xs2[o] = sum_i y_s[i]^2 * w2[i,o],  w2 = sum_k weight[o,i,k]^2
  out = conv / sqrt(xs2 + 1e-8 * Ci * K * K) + bias

Schedule (from trace analysis of the 85.4us baseline):
- exec window = first kernel-body instr -> end of framework teardown
  (~8.6us fixed), so only stream-start latency, stream density, and the
  last-output-landed time matter.
- Two DMA rings: ring A (sync queue) carries x (x0 split in half-tiles so
  modulation can start after 128KB) then the jo=2 weight tiles; ring B
  (scalar queue) carries weights in first-use order, wt(0,0) split in 3
  k-chunks. Same-queue transfers serialize FIFO; the two rings share HBM.
  yb goes on the gpsimd queue. No dummy-read throttles (they never
  serialized anything: Tile orders by dataflow, not emission).
- PE p-state: HAM grants full speed only ~4.5us after *sustained* matmul
  activity; a warmup burst that ends before the real stream leaves the
  grant untriggered. So the warmup (small 64-row matmuls) is sized to run
  gapless from engine start until the first conv matmul's inputs land.
- jo=0..2 interleave (j, half): each j-block feeds both 16-row halves
  back-to-back, so weights/x for j+1 are needed 3.8us (not 1.9us) in.
  jo=3 runs half-major so (3,0)'s epilogue+DMA overlap (3,1)'s matmuls
  and only one 512-col epilogue sits on the tail; that one is split
  scalar/vector with its two output DMAs on different queues.
- Output dtype bf16 (host upcasts): halves output DMA bytes; ~0.2% rms
  rounding vs the 2e-2 gate.
"""

import numpy as np
import ml_dtypes

import concourse.mybir as mybir
from concourse import bacc
from concourse.tile import TileContext
from concourse.bass_utils import run_bass_kernel_spmd

B = 8
CI = 512
CO = 512
H = W = 32
KK = 9  # 3x3
NCI = CI // 128
NCO = CO // 128
HWPAD = 34
EPS_EFF = 1e-8 * CI * KK  # demod eps compensated for unscaled weights
N_WARM = 36  # 64-row warmup matmuls: cover engine start..stream start, gapless

F32 = mybir.dt.float32
BF16 = mybir.dt.bfloat16
AF = mybir.ActivationFunctionType
MULT = mybir.AluOpType.mult
ADD = mybir.AluOpType.add


def build_nc():
    nc = bacc.Bacc("TRN2", target_bir_lowering=False, debug=False)

    x_ext = nc.declare_dram_parameter("x", [NCI, 128, H, W], BF16, isOutput=False)
    # cols 0..3 = y_s per ci-tile, cols 4..7 = bias per co-tile
    yb_ext = nc.declare_dram_parameter("yb", [128, 2 * NCI], F32, isOutput=False)
    # [jci, jco, ci_p, k(9)+w2(1), co_c] bf16
    wt_ext = nc.declare_dram_parameter(
        "wt", [NCI, NCO, 128, KK + 1, 128], BF16, isOutput=False
    )
    out_ext = nc.declare_dram_parameter("out", [NCO, 128, H * W], BF16, isOutput=True)

    with TileContext(nc) as tc:
        with (
            tc.tile_pool(name="singles", bufs=1) as singles,
            tc.tile_pool(name="wts", bufs=1) as wts,
            tc.tile_pool(name="pads", bufs=1) as pads,
            tc.tile_pool(name="xin", bufs=4) as xin,
            tc.tile_pool(name="outs", bufs=3) as outs,
            tc.tile_pool(name="cps", bufs=6, space="PSUM") as cps,
            tc.tile_pool(name="dps", bufs=1, space="PSUM") as dps,
            tc.tile_pool(name="wps", bufs=1, space="PSUM") as wps,
        ):
            xt = [
                xin.tile([128, H, W], BF16, tag=f"x{j}", name=f"xt{j}")
                for j in range(NCI)
            ]
            yb_sb = singles.tile([128, 2 * NCI], F32)
            wt_sb = [
                [
                    wts.tile(
                        [128, KK + 1, 128], BF16, tag=f"wt{j}_{q}", name=f"wt{j}_{q}"
                    )
                    for q in range(NCO)
                ]
                for j in range(NCI)
            ]

            # ---- ring C (gpsimd queue): the T0-critical trio + x0b. This
            # engine's body dispatches ~1us before sync/scalar, so its ring
            # wakes first; everything the first conv matmuls need rides it.
            nc.gpsimd.dma_start(out=yb_sb, in_=yb_ext[:, :])
            nc.gpsimd.dma_start(out=xt[0][:, 0:16, :], in_=x_ext[0][:, 0:16, :])
            nc.gpsimd.dma_start(
                out=wt_sb[0][0][:, 0:3, :], in_=wt_ext[0, 0][:, 0:3, :]
            )
            nc.gpsimd.dma_start(out=xt[0][:, 16:32, :], in_=x_ext[0][:, 16:32, :])
            # ---- ring A (sync queue): rest of x, jo=2 weights, later the
            # output chunks. FIFO within the queue.
            for j in range(1, NCI):
                nc.sync.dma_start(out=xt[j], in_=x_ext[j])
            for j in range(NCI):
                nc.sync.dma_start(out=wt_sb[j][2], in_=wt_ext[j, 2])
            # ---- ring B (scalar queue): weights in first-use order; early
            # tiles split in k-chunks so deadlines are met incrementally.
            nc.scalar.dma_start(out=wt_sb[0][0][:, 3:6, :], in_=wt_ext[0, 0][:, 3:6, :])
            nc.scalar.dma_start(
                out=wt_sb[0][0][:, 6 : KK + 1, :], in_=wt_ext[0, 0][:, 6 : KK + 1, :]
            )
            nc.scalar.dma_start(out=wt_sb[1][0][:, 0:3, :], in_=wt_ext[1, 0][:, 0:3, :])
            nc.scalar.dma_start(
                out=wt_sb[1][0][:, 3 : KK + 1, :], in_=wt_ext[1, 0][:, 3 : KK + 1, :]
            )
            for j in range(2, NCI):
                nc.scalar.dma_start(out=wt_sb[j][0], in_=wt_ext[j, 0])
            for q in (1, 3):
                for j in range(NCI):
                    nc.scalar.dma_start(out=wt_sb[j][q], in_=wt_ext[j, q])

            # ---- PE warmup: gapless small matmuls from engine start until
            # the real stream's inputs land, so the HAM full-speed grant
            # (triggered by sustained activity) arrives early.
            warm_lhs = singles.tile([128, 1], BF16)
            nc.vector.memset(warm_lhs, 1.0)
            warm_rhs = singles.tile([128, 64], BF16)
            nc.vector.memset(warm_rhs, 0.5)
            warm_ps = wps.tile([1, 64], F32)
            for i in range(N_WARM):
                nc.tensor.matmul(
                    out=warm_ps,
                    lhsT=warm_lhs,
                    rhs=warm_rhs,
                    start=(i == 0),
                    stop=(i == N_WARM - 1),
                )

            # ---- zero-padded modulated input (bf16), border-only memsets
            pad_sb = []
            for j in range(NCI):
                p = pads.tile([128, HWPAD, HWPAD], BF16, tag=f"pad{j}")
                nc.gpsimd.memset(p[:, 0, :], 0.0)
                nc.gpsimd.memset(p[:, HWPAD - 1, :], 0.0)
                nc.gpsimd.memset(p[:, 1 : HWPAD - 1, 0], 0.0)
                nc.gpsimd.memset(p[:, 1 : HWPAD - 1, HWPAD - 1], 0.0)
                pad_sb.append(p)

            def modulate(j, r0, r1):
                nc.vector.tensor_scalar(
                    out=pad_sb[j][:, 1 + r0 : 1 + r1, 1 : W + 1],
                    in0=xt[j][:, r0:r1, :],
                    scalar1=yb_sb[:, j : j + 1],
                    scalar2=None,
                    op0=MULT,
                )

            modulate(0, 0, 16)
            modulate(0, 16, 32)
            modulate(1, 0, 32)
            ys2_sb = singles.tile([128, NCI], BF16)
            nc.vector.tensor_mul(ys2_sb, yb_sb[:, 0:NCI], yb_sb[:, 0:NCI])
            eps_sb = singles.tile([128, 1], F32)
            nc.vector.memset(eps_sb, EPS_EFF)
            modulate(2, 0, 32)
            modulate(3, 0, 32)

            def wt_slice(j, jo, k):
                return wt_sb[j][jo][:, k, :]

            def conv_block_interleaved(jo):
                # both 16-row halves per j-block: j+1's inputs needed 3.8us in
                psA = cps.tile([128, 512], F32, tag="ps")
                psB = cps.tile([128, 512], F32, tag="ps")
                cnt = [0, 0]
                for j in range(NCI):
                    if jo == 0 and j == 0:
                        # kh=2 rows and the weight k6..8 chunk arrive last;
                        # run both halves' k0..5 first to relax deadlines
                        seq = (
                            [(0, k) for k in range(6)]
                            + [(1, k) for k in range(6)]
                            + [(0, k) for k in range(6, KK)]
                            + [(1, k) for k in range(6, KK)]
                        )
                    else:
                        seq = [(h, k) for h in (0, 1) for k in range(KK)]
                    for h, k in seq:
                        ps = psA if h == 0 else psB
                        kh, kw = divmod(k, 3)
                        rhs = pad_sb[j][:, kh + 16 * h : kh + 16 * h + 16, kw : kw + W]
                        nc.tensor.matmul(
                            out=ps,
                            lhsT=wt_slice(j, jo, k),
                            rhs=rhs,
                            start=(cnt[h] == 0),
                            stop=(cnt[h] == NCI * KK - 1),
                        )
                        cnt[h] += 1
                return psA, psB

            def conv_group_rows(jo, r0, nrows):
                # one accumulation group covering output rows r0..r0+nrows
                ps = cps.tile([128, nrows * W], F32, tag="ps")
                idx = 0
                for j in range(NCI):
                    for k in range(KK):
                        kh, kw = divmod(k, 3)
                        rhs = pad_sb[j][:, kh + r0 : kh + r0 + nrows, kw : kw + W]
                        nc.tensor.matmul(
                            out=ps,
                            lhsT=wt_slice(j, jo, k),
                            rhs=rhs,
                            start=(idx == 0),
                            stop=(idx == NCI * KK - 1),
                        )
                        idx += 1
                return ps

            xs2_ps = dps.tile([128, NCO], F32)
            rs_sb = singles.tile([128, NCO], F32)

            def epilogue(ps, jo, half, eng):
                ot = outs.tile([128, 512], BF16, tag="ot")
                if eng == "s":
                    nc.scalar.activation(
                        out=ot,
                        in_=ps,
                        func=AF.Identity,
                        bias=yb_sb[:, NCI + jo : NCI + jo + 1],
                        scale=rs_sb[:, jo : jo + 1],
                    )
                else:
                    nc.vector.tensor_scalar(
                        out=ot,
                        in0=ps,
                        scalar1=rs_sb[:, jo : jo + 1],
                        scalar2=yb_sb[:, NCI + jo : NCI + jo + 1],
                        op0=MULT,
                        op1=ADD,
                    )
                nc.sync.dma_start(
                    out=out_ext[jo, :, half * 512 : (half + 1) * 512], in_=ot
                )

            def epilogue_rows(ps, jo, c0, ncols, eng):
                ot = outs.tile([128, ncols], BF16, tag=f"otr{c0}", name=f"otr{c0}")
                if eng == "s":
                    nc.scalar.activation(
                        out=ot,
                        in_=ps,
                        func=AF.Identity,
                        bias=yb_sb[:, NCI + jo : NCI + jo + 1],
                        scale=rs_sb[:, jo : jo + 1],
                    )
                else:
                    nc.vector.tensor_scalar(
                        out=ot,
                        in0=ps,
                        scalar1=rs_sb[:, jo : jo + 1],
                        scalar2=yb_sb[:, NCI + jo : NCI + jo + 1],
                        op0=MULT,
                        op1=ADD,
                    )
                nc.sync.dma_start(out=out_ext[jo, :, c0 : c0 + ncols], in_=ot)

            # ---- stream: jo0, jo1 | demod | eps | jo2 | jo3 row-split
            b0A, b0B = conv_block_interleaved(0)
            b1A, b1B = conv_block_interleaved(1)
            for jo in range(NCO):
                for j in range(NCI):
                    nc.tensor.matmul(
                        out=xs2_ps[:, jo : jo + 1],
                        lhsT=wt_slice(j, jo, KK),
                        rhs=ys2_sb[:, j : j + 1],
                        start=(j == 0),
                        stop=(j == NCI - 1),
                    )
            nc.scalar.activation(out=rs_sb, in_=xs2_ps, func=AF.Sqrt, bias=eps_sb)
            nc.vector.reciprocal(out=rs_sb, in_=rs_sb)
            # keep the warm-up matmuls live; emitted here so vector's read
            # doesn't land on the drain-critical end of its queue
            warm_sink = singles.tile([1, 1], F32)
            nc.vector.tensor_copy(out=warm_sink, in_=warm_ps[0:1, 0:1])
            epilogue(b0A, 0, 0, "s")
            epilogue(b0B, 0, 1, "v")
            epilogue(b1A, 1, 0, "s")
            epilogue(b1B, 1, 1, "v")
            b2A, b2B = conv_block_interleaved(2)
            epilogue(b2A, 2, 0, "s")
            epilogue(b2B, 2, 1, "v")
            ps30 = conv_group_rows(3, 0, 16)
            epilogue(ps30, 3, 0, "s")
            # last 16 rows as two 8-row groups: (3,1a)'s epilogue+DMA run
            # under (3,1b)'s matmuls, leaving a single 256-col tail
            ps31a = conv_group_rows(3, 16, 8)
            epilogue_rows(ps31a, 3, 512, 256, "s")
            ps31b = conv_group_rows(3, 24, 8)
            epilogue_rows(ps31b, 3, 768, 256, "s")
    nc.compile()
    return nc


_NC_CACHE = None


def _get_nc():
    global _NC_CACHE
    if _NC_CACHE is None:
        _NC_CACHE = build_nc()
    return _NC_CACHE


def _prep_inputs(x, y_s, weight, bias):
    # [co, ci, kh, kw] -> [k, ci, co]; append w2 = sum_k wt^2 as slot 9;
    # then tile to [jci, jco, ci_p, 10, co_c] bf16 contiguous.
    wt9 = weight.transpose(2, 3, 1, 0).reshape(KK, CI, CO)
    w2 = (wt9.astype(np.float64) ** 2).sum(axis=0).astype(np.float32)
    full = np.concatenate([wt9, w2[None]], axis=0)  # [10, ci, co]
    wtq = np.ascontiguousarray(
        full.reshape(KK + 1, NCI, 128, NCO, 128).transpose(1, 3, 2, 0, 4)
    ).astype(ml_dtypes.bfloat16)
    in_maps = []
    for b in range(B):
        yb = np.empty((128, 2 * NCI), np.float32)
        yb[:, :NCI] = y_s[b].reshape(NCI, 128).T
        yb[:, NCI:] = bias.reshape(NCO, 128).T
        in_maps.append(
            {
                "x": np.ascontiguousarray(x[b].reshape(NCI, 128, H, W)).astype(
                    ml_dtypes.bfloat16
                ),
                "yb": yb,
                "wt": wtq,
            }
        )
    return in_maps


def _install_trace_support():
    """Dev-only: register the axon NTFF profiling hook + disable the
    remote artifact upload so trace=True works in this container."""
    import sys
    import types

    import concourse.bass_utils as bu

    bu.upload_artifacts = lambda tmpdir: "local://" + str(tmpdir)
    if "antenv.axon_hooks" in sys.modules:
        return
    try:
        from trn_agent_boot.trn_boot import _ntff_profile_via_ctypes

        hook = _ntff_profile_via_ctypes("/opt/axon/libaxon_pjrt.so")
    except Exception:
        return
    mod = types.ModuleType("antenv.axon_hooks")
    mod.get_axon_ntff_profile_hook = lambda: hook
    mod.set_axon_ntff_profile_hook = lambda h: None
    sys.modules["antenv.axon_hooks"] = mod


def run(x, y_s, weight, bias, trace=False, tmpdir=None):
    nc = _get_nc()
    if trace:
        _install_trace_support()
    in_maps = _prep_inputs(x, y_s, weight, bias)
    res = run_bass_kernel_spmd(
        nc, in_maps, core_ids=list(range(B)), trace=trace, tmpdir=tmpdir
    )
    out = np.stack(
        [
            np.asarray(res.results[b]["out"]).astype(np.float32).reshape(CO, H, W)
            for b in range(B)
        ]
    )
    return out, res


def kernel(x, y_s, weight, bias):
    out, _ = run(
        np.asarray(x, dtype=np.float32),
        np.asarray(y_s, dtype=np.float32),
        np.asarray(weight, dtype=np.float32),
        np.asarray(bias, dtype=np.float32),
    )
    return out


# revision 11
# speedup vs baseline: 1.0806x; 1.0806x over previous
"""StyleGAN2-style modulated 3x3 conv (B=8, Ci=Co=512, H=W=32) on 8 TRN2 NeuronCores.

Sharding: data-parallel over batch, one sample per core. Per core the conv
is 9 shifted matmuls over a zero-padded 34x34 image in SBUF, contracting
over Ci in 128-chunks with fp32 PSUM accumulation; compute dtype bf16.

Math (per sample b, with s = (Ci*K*K)**-0.5 folded out of both the conv
and the demod norm so the weights can be used unscaled):
  conv = conv2d(x * y_s, weight)                     # raw, no s
  xs2[o] = sum_i y_s[i]^2 * w2[i,o],  w2 = sum_k weight[o,i,k]^2
  out = conv / sqrt(xs2 + 1e-8 * Ci * K * K) + bias

Schedule (from trace analysis of the 85.4us baseline):
- exec window = first kernel-body instr -> end of framework teardown
  (~8.6us fixed), so only stream-start latency, stream density, and the
  last-output-landed time matter.
- Two DMA rings: ring A (sync queue) carries x (x0 split in half-tiles so
  modulation can start after 128KB) then the jo=2 weight tiles; ring B
  (scalar queue) carries weights in first-use order, wt(0,0) split in 3
  k-chunks. Same-queue transfers serialize FIFO; the two rings share HBM.
  yb goes on the gpsimd queue. No dummy-read throttles (they never
  serialized anything: Tile orders by dataflow, not emission).
- PE p-state: HAM grants full speed only ~4.5us after *sustained* matmul
  activity; a warmup burst that ends before the real stream leaves the
  grant untriggered. So the warmup (small 64-row matmuls) is sized to run
  gapless from engine start until the first conv matmul's inputs land.
- jo=0..2 interleave (j, half): each j-block feeds both 16-row halves
  back-to-back, so weights/x for j+1 are needed 3.8us (not 1.9us) in.
  jo=3 runs half-major so (3,0)'s epilogue+DMA overlap (3,1)'s matmuls
  and only one 512-col epilogue sits on the tail; that one is split
  scalar/vector with its two output DMAs on different queues.
- Output dtype bf16 (host upcasts): halves output DMA bytes; ~0.2% rms
  rounding vs the 2e-2 gate.
"""

import numpy as np
import ml_dtypes

import concourse.mybir as mybir
from concourse import bacc
from concourse.tile import TileContext
from concourse.bass_utils import run_bass_kernel_spmd

B = 8
CI = 512
CO = 512
H = W = 32
KK = 9  # 3x3
NCI = CI // 128
NCO = CO // 128
HWPAD = 34
EPS_EFF = 1e-8 * CI * KK  # demod eps compensated for unscaled weights
N_WARM = 52  # 64-row warmup matmuls: cover engine start..stream start, gapless

F32 = mybir.dt.float32
BF16 = mybir.dt.bfloat16
AF = mybir.ActivationFunctionType
MULT = mybir.AluOpType.mult
ADD = mybir.AluOpType.add


def build_nc():
    nc = bacc.Bacc("TRN2", target_bir_lowering=False, debug=False)

    x_ext = nc.declare_dram_parameter("x", [NCI, 128, H, W], BF16, isOutput=False)
    # cols 0..3 = y_s per ci-tile, cols 4..7 = bias per co-tile
    yb_ext = nc.declare_dram_parameter("yb", [128, 2 * NCI], F32, isOutput=False)
    # [jci, jco, ci_p, k(9)+w2(1), co_c] bf16
    wt_ext = nc.declare_dram_parameter(
        "wt", [NCI, NCO, 128, KK + 1, 128], BF16, isOutput=False
    )
    out_ext = nc.declare_dram_parameter("out", [NCO, 128, H * W], BF16, isOutput=True)

    with TileContext(nc) as tc:
        with (
            tc.tile_pool(name="singles", bufs=1) as singles,
            tc.tile_pool(name="wts", bufs=1) as wts,
            tc.tile_pool(name="pads", bufs=1) as pads,
            tc.tile_pool(name="xin", bufs=4) as xin,
            tc.tile_pool(name="outs", bufs=3) as outs,
            tc.tile_pool(name="cps", bufs=6, space="PSUM") as cps,
            tc.tile_pool(name="dps", bufs=1, space="PSUM") as dps,
            tc.tile_pool(name="wps", bufs=1, space="PSUM") as wps,
        ):
            xt = [
                xin.tile([128, H, W], BF16, tag=f"x{j}", name=f"xt{j}")
                for j in range(NCI)
            ]
            yb_sb = singles.tile([128, 2 * NCI], F32)
            wt_sb = [
                [
                    wts.tile(
                        [128, KK + 1, 128], BF16, tag=f"wt{j}_{q}", name=f"wt{j}_{q}"
                    )
                    for q in range(NCO)
                ]
                for j in range(NCI)
            ]

            # ---- ring A (sync queue): yb + x in first-use order (x0 split),
            # then jo=2 weights, later the output chunks. FIFO within queue.
            nc.sync.dma_start(out=yb_sb, in_=yb_ext[:, :])
            nc.sync.dma_start(out=xt[0][:, 0:16, :], in_=x_ext[0][:, 0:16, :])
            nc.sync.dma_start(out=xt[0][:, 16:32, :], in_=x_ext[0][:, 16:32, :])
            for j in range(1, NCI):
                nc.sync.dma_start(out=xt[j], in_=x_ext[j])
            for j in range(NCI):
                nc.sync.dma_start(out=wt_sb[j][2], in_=wt_ext[j, 2])
            # ---- ring B (scalar queue): weights in first-use order; early
            # tiles split in k-chunks so deadlines are met incrementally.
            nc.scalar.dma_start(out=wt_sb[0][0][:, 0:3, :], in_=wt_ext[0, 0][:, 0:3, :])
            nc.scalar.dma_start(out=wt_sb[0][0][:, 3:6, :], in_=wt_ext[0, 0][:, 3:6, :])
            nc.scalar.dma_start(
                out=wt_sb[0][0][:, 6 : KK + 1, :], in_=wt_ext[0, 0][:, 6 : KK + 1, :]
            )
            nc.scalar.dma_start(out=wt_sb[1][0][:, 0:3, :], in_=wt_ext[1, 0][:, 0:3, :])
            nc.scalar.dma_start(
                out=wt_sb[1][0][:, 3 : KK + 1, :], in_=wt_ext[1, 0][:, 3 : KK + 1, :]
            )
            for j in range(2, NCI):
                nc.scalar.dma_start(out=wt_sb[j][0], in_=wt_ext[j, 0])
            for q in (1, 3):
                for j in range(NCI):
                    nc.scalar.dma_start(out=wt_sb[j][q], in_=wt_ext[j, q])

            # ---- PE warmup: gapless small matmuls from engine start until
            # the real stream's inputs land, so the HAM full-speed grant
            # (triggered by sustained activity) arrives early.
            warm_lhs = singles.tile([128, 1], BF16)
            nc.vector.memset(warm_lhs, 1.0)
            warm_rhs = singles.tile([128, 64], BF16)
            nc.vector.memset(warm_rhs, 0.5)
            warm_ps = wps.tile([1, 64], F32)
            for i in range(N_WARM):
                nc.tensor.matmul(
                    out=warm_ps,
                    lhsT=warm_lhs,
                    rhs=warm_rhs,
                    start=(i == 0),
                    stop=(i == N_WARM - 1),
                )

            # ---- zero-padded modulated input (bf16), border-only memsets
            pad_sb = []
            for j in range(NCI):
                p = pads.tile([128, HWPAD, HWPAD], BF16, tag=f"pad{j}")
                nc.gpsimd.memset(p[:, 0, :], 0.0)
                nc.gpsimd.memset(p[:, HWPAD - 1, :], 0.0)
                nc.gpsimd.memset(p[:, 1 : HWPAD - 1, 0], 0.0)
                nc.gpsimd.memset(p[:, 1 : HWPAD - 1, HWPAD - 1], 0.0)
                pad_sb.append(p)

            def modulate(j, r0, r1):
                nc.vector.tensor_scalar(
                    out=pad_sb[j][:, 1 + r0 : 1 + r1, 1 : W + 1],
                    in0=xt[j][:, r0:r1, :],
                    scalar1=yb_sb[:, j : j + 1],
                    scalar2=None,
                    op0=MULT,
                )

            modulate(0, 0, 16)
            modulate(0, 16, 32)
            modulate(1, 0, 32)
            ys2_sb = singles.tile([128, NCI], BF16)
            nc.vector.tensor_mul(ys2_sb, yb_sb[:, 0:NCI], yb_sb[:, 0:NCI])
            eps_sb = singles.tile([128, 1], F32)
            nc.vector.memset(eps_sb, EPS_EFF)
            modulate(2, 0, 32)
            modulate(3, 0, 32)

            def wt_slice(j, jo, k):
                return wt_sb[j][jo][:, k, :]

            def conv_block_interleaved(jo):
                # both 16-row halves per j-block: j+1's inputs needed 3.8us in
                psA = cps.tile([128, 512], F32, tag="ps")
                psB = cps.tile([128, 512], F32, tag="ps")
                cnt = [0, 0]
                for j in range(NCI):
                    if jo == 0 and j == 0:
                        # kh=2 rows and the weight k6..8 chunk arrive last;
                        # run both halves' k0..5 first to relax deadlines
                        seq = (
                            [(0, k) for k in range(6)]
                            + [(1, k) for k in range(6)]
                            + [(0, k) for k in range(6, KK)]
                            + [(1, k) for k in range(6, KK)]
                        )
                    else:
                        seq = [(h, k) for h in (0, 1) for k in range(KK)]
                    for h, k in seq:
                        ps = psA if h == 0 else psB
                        kh, kw = divmod(k, 3)
                        rhs = pad_sb[j][:, kh + 16 * h : kh + 16 * h + 16, kw : kw + W]
                        nc.tensor.matmul(
                            out=ps,
                            lhsT=wt_slice(j, jo, k),
                            rhs=rhs,
                            start=(cnt[h] == 0),
                            stop=(cnt[h] == NCI * KK - 1),
                        )
                        cnt[h] += 1
                return psA, psB

            def conv_group_rows(jo, r0, nrows):
                # one accumulation group covering output rows r0..r0+nrows
                ps = cps.tile([128, nrows * W], F32, tag="ps")
                idx = 0
                for j in range(NCI):
                    for k in range(KK):
                        kh, kw = divmod(k, 3)
                        rhs = pad_sb[j][:, kh + r0 : kh + r0 + nrows, kw : kw + W]
                        nc.tensor.matmul(
                            out=ps,
                            lhsT=wt_slice(j, jo, k),
                            rhs=rhs,
                            start=(idx == 0),
                            stop=(idx == NCI * KK - 1),
                        )
                        idx += 1
                return ps

            xs2_ps = dps.tile([128, NCO], F32)
            rs_sb = singles.tile([128, NCO], F32)

            def epilogue(ps, jo, half, eng):
                ot = outs.tile([128, 512], BF16, tag="ot")
                if eng == "s":
                    nc.scalar.activation(
                        out=ot,
                        in_=ps,
                        func=AF.Identity,
                        bias=yb_sb[:, NCI + jo : NCI + jo + 1],
                        scale=rs_sb[:, jo : jo + 1],
                    )
                else:
                    nc.vector.tensor_scalar(
                        out=ot,
                        in0=ps,
                        scalar1=rs_sb[:, jo : jo + 1],
                        scalar2=yb_sb[:, NCI + jo : NCI + jo + 1],
                        op0=MULT,
                        op1=ADD,
                    )
                nc.sync.dma_start(
                    out=out_ext[jo, :, half * 512 : (half + 1) * 512], in_=ot
                )

            def epilogue_rows(ps, jo, c0, ncols, eng):
                ot = outs.tile([128, ncols], BF16, tag=f"otr{c0}", name=f"otr{c0}")
                if eng == "s":
                    nc.scalar.activation(
                        out=ot,
                        in_=ps,
                        func=AF.Identity,
                        bias=yb_sb[:, NCI + jo : NCI + jo + 1],
                        scale=rs_sb[:, jo : jo + 1],
                    )
                else:
                    nc.vector.tensor_scalar(
                        out=ot,
                        in0=ps,
                        scalar1=rs_sb[:, jo : jo + 1],
                        scalar2=yb_sb[:, NCI + jo : NCI + jo + 1],
                        op0=MULT,
                        op1=ADD,
                    )
                nc.sync.dma_start(out=out_ext[jo, :, c0 : c0 + ncols], in_=ot)

            # ---- stream: jo0, jo1 | demod | eps | jo2 | jo3 row-split
            b0A, b0B = conv_block_interleaved(0)
            b1A, b1B = conv_block_interleaved(1)
            for jo in range(NCO):
                for j in range(NCI):
                    nc.tensor.matmul(
                        out=xs2_ps[:, jo : jo + 1],
                        lhsT=wt_slice(j, jo, KK),
                        rhs=ys2_sb[:, j : j + 1],
                        start=(j == 0),
                        stop=(j == NCI - 1),
                    )
            nc.scalar.activation(out=rs_sb, in_=xs2_ps, func=AF.Sqrt, bias=eps_sb)
            nc.vector.reciprocal(out=rs_sb, in_=rs_sb)
            # keep the warm-up matmuls live; emitted here so vector's read
            # doesn't land on the drain-critical end of its queue
            warm_sink = singles.tile([1, 1], F32)
            nc.vector.tensor_copy(out=warm_sink, in_=warm_ps[0:1, 0:1])
            epilogue(b0A, 0, 0, "s")
            epilogue(b0B, 0, 1, "v")
            epilogue(b1A, 1, 0, "s")
            epilogue(b1B, 1, 1, "v")
            b2A, b2B = conv_block_interleaved(2)
            epilogue(b2A, 2, 0, "s")
            epilogue(b2B, 2, 1, "v")
            ps30 = conv_group_rows(3, 0, 16)
            epilogue(ps30, 3, 0, "s")
            # last 16 rows as two 8-row groups: (3,1a)'s epilogue+DMA run
            # under (3,1b)'s matmuls, leaving a single 256-col tail
            ps31a = conv_group_rows(3, 16, 8)
            epilogue_rows(ps31a, 3, 512, 256, "s")
            ps31b = conv_group_rows(3, 24, 8)
            epilogue_rows(ps31b, 3, 768, 256, "s")
    nc.compile()
    return nc


_NC_CACHE = None


def _get_nc():
    global _NC_CACHE
    if _NC_CACHE is None:
        _NC_CACHE = build_nc()
    return _NC_CACHE


def _prep_inputs(x, y_s, weight, bias):
    # [co, ci, kh, kw] -> [k, ci, co]; append w2 = sum_k wt^2 as slot 9;
    # then tile to [jci, jco, ci_p, 10, co_c] bf16 contiguous.
    wt9 = weight.transpose(2, 3, 1, 0).reshape(KK, CI, CO)
    w2 = (wt9.astype(np.float64) ** 2).sum(axis=0).astype(np.float32)
    full = np.concatenate([wt9, w2[None]], axis=0)  # [10, ci, co]
    wtq = np.ascontiguousarray(
        full.reshape(KK + 1, NCI, 128, NCO, 128).transpose(1, 3, 2, 0, 4)
    ).astype(ml_dtypes.bfloat16)
    in_maps = []
    for b in range(B):
        yb = np.empty((128, 2 * NCI), np.float32)
        yb[:, :NCI] = y_s[b].reshape(NCI, 128).T
        yb[:, NCI:] = bias.reshape(NCO, 128).T
        in_maps.append(
            {
                "x": np.ascontiguousarray(x[b].reshape(NCI, 128, H, W)).astype(
                    ml_dtypes.bfloat16
                ),
                "yb": yb,
                "wt": wtq,
            }
        )
    return in_maps


def _install_trace_support():
    """Dev-only: register the axon NTFF profiling hook + disable the
    remote artifact upload so trace=True works in this container."""
    import sys
    import types

    import concourse.bass_utils as bu

    bu.upload_artifacts = lambda tmpdir: "local://" + str(tmpdir)
    if "antenv.axon_hooks" in sys.modules:
        return
    try:
        from trn_agent_boot.trn_boot import _ntff_profile_via_ctypes

        hook = _ntff_profile_via_ctypes("/opt/axon/libaxon_pjrt.so")
    except Exception:
        return
    mod = types.ModuleType("antenv.axon_hooks")
    mod.get_axon_ntff_profile_hook = lambda: hook
    mod.set_axon_ntff_profile_hook = lambda h: None
    sys.modules["antenv.axon_hooks"] = mod


def run(x, y_s, weight, bias, trace=False, tmpdir=None):
    nc = _get_nc()
    if trace:
        _install_trace_support()
    in_maps = _prep_inputs(x, y_s, weight, bias)
    res = run_bass_kernel_spmd(
        nc, in_maps, core_ids=list(range(B)), trace=trace, tmpdir=tmpdir
    )
    out = np.stack(
        [
            np.asarray(res.results[b]["out"]).astype(np.float32).reshape(CO, H, W)
            for b in range(B)
        ]
    )
    return out, res


def kernel(x, y_s, weight, bias):
    out, _ = run(
        np.asarray(x, dtype=np.float32),
        np.asarray(y_s, dtype=np.float32),
        np.asarray(weight, dtype=np.float32),
        np.asarray(bias, dtype=np.float32),
    )
    return out
